# revision 7
# baseline (speedup 1.0000x reference)
"""MoE GemmaMLP (top-2 of 8 experts + shared expert) on 8 trn2 NeuronCores.

Expert-parallel packing (seed-0 load: 16 two-pair single-expert weight-stream
groups, one 2+2 pair config per core, + 2 leftover pairs run tensor-parallel
over I on all cores; shared expert split over I, batch-replicated), bf16
end-to-end:
  - all matmuls bf16 (tolerance 2e-2 >> bf16's ~4.7e-3 measured): halves
    every DMA transfer and the SBUF footprint vs f32/f32r. HW-measured MM
    issue rate at N=512 is ~256 ns/MM for bf16 and f32r alike (slope-timed
    microbench), and LDWEIGHTS is hidden by the PE reorder window (4x weight
    reuse saves <10 ns/MM), so the dtype change buys bandwidth, not FLOPs.
  - down projection is weight-stationary: out^T[h,s] = sum_i wd[i,h]^T aT
    with i as the outer (PSUM-accumulated) loop; outputs land transposed
    [H, S] and the host transposes during the weighted gather for free.
    PSUM drains drop 2x vs activation-stationary.
  - weights stream as per-chunk slabs (1-2KB contiguous runs, ~0.25-1MB per
    DMA) alternating between the SP and ACT HWDGE rings.
  - PSUM: 4 banks gate/up + 4 banks down (the old down path used one bank,
    serializing every drain).
  - xt pool holds 4 slots (2 live + 2 prefetch) so the next group's inputs
    load during the current group's compute; shared-expert slabs are loaded
    after the first expert group's DMAs so the first matmul starts ~15us
    earlier.
Per-core: 4800 matmuls x 256 ns =~ 1.23 ms PE floor (HW-measured flat
per-instruction rate; unaffected by accumulation/bank patterns or weight
reuse). Measured 1.24 ms steady-state (interleaved pipelined nreps-slope
timing), vs 1.86 ms for the f32r baseline. A per-phase f32r variant
measured slower (1.39 ms): f32r's ~20ns/MM issue-rate edge is outweighed
by doubled weight HBM traffic.
"""

import os
import numpy as np
import ml_dtypes

import concourse.mybir as mybir
import concourse.tile as tile
from concourse import bacc
from concourse.bass_utils import run_bass_kernel_spmd

B, S, H, I, E = 32, 256, 1024, 4096, 8
TOP_K = 2
NUM_MOE_LAYERS = 12
NCORES = 8
IS = I // NCORES          # shared-expert intermediate slice per core
HT = H // 128             # h-tiles
P = 128
S2 = 2 * S                # moving columns per pair (2 batches)
CHUNK = 4                 # i-tiles per weight-stream / down-accum chunk

F32 = mybir.dt.float32
BF = mybir.dt.bfloat16
NPBF = ml_dtypes.bfloat16
GELU = mybir.ActivationFunctionType.Gelu_apprx_tanh

GROUPS = ((2, 32), (2, 32), (1, 4), (1, 4))  # (pairs, i-tiles) per slot


def _group(nc, pools, xt_rows, wg_d, wu_d, wd_d, out_rows, n_pairs, ni,
           res_w=None, ring_flip=0):
    """One weight-stream group: n_pairs pairs sharing one expert's weights.

    xt_rows: 2*n_pairs DRAM APs [H, S] (bf16)
    out_rows: 2*n_pairs DRAM APs [H, S] (bf16, transposed output)
    res_w: optional (wg_t, wu_t, wd_t) resident SBUF slabs (shared expert);
        layout identical to the streamed chunk slabs with chn == ni.
    """
    xt_p, psgu, psd_p, tmp_p, at_p = (
        pools[k] for k in ("xt", "psgu", "psd", "tmp", "aT"))
    rings = (nc.sync, nc.scalar)

    xt_t = []
    for pr in range(n_pairs):
        t = xt_p.tile([P, HT * S2], BF, tag="xt", name=f"xt{pr}")
        v = t[:].rearrange("p (t c) -> p t c", c=S2)
        rings[ring_flip].dma_start(
            v[:, :, 0:S], xt_rows[2 * pr].rearrange("(t p) s -> p t s", p=P))
        rings[1 - ring_flip].dma_start(
            v[:, :, S:S2],
            xt_rows[2 * pr + 1].rearrange("(t p) s -> p t s", p=P))
        xt_t.append(t)

    n_chunks = (ni + CHUNK - 1) // CHUNK
    acc = None
    if n_chunks > 1:
        acc = pools["acc"].tile([P, HT * n_pairs * S2], F32, tag="acc")
    stage = pools["stage"].tile([P, HT * n_pairs * S2], BF, tag="stage")
    DHT = 2 if n_pairs == 2 else 4  # down ht-tiles per PSUM pass (4 banks)

    for ci in range(n_chunks):
        c0 = ci * CHUNK
        chn = min(CHUNK, ni - c0)
        if res_w is not None:
            wg_t, wu_t, wd_t = res_w
            wcols, woff = ni * P, c0 * P
        else:
            wcols, woff = chn * P, 0
            wg_t = pools["wg"].tile([P, HT * chn * P], BF, tag="wg")
            rings[(ci + ring_flip) % 2].dma_start(
                wg_t[:].rearrange("p (t c) -> p t c", c=chn * P),
                wg_d.rearrange("(t p) c -> p t c", p=P)
                [:, :, c0 * P:(c0 + chn) * P])
            wu_t = pools["wu"].tile([P, HT * chn * P], BF, tag="wu")
            rings[(ci + 1 + ring_flip) % 2].dma_start(
                wu_t[:].rearrange("p (t c) -> p t c", c=chn * P),
                wu_d.rearrange("(t p) c -> p t c", p=P)
                [:, :, c0 * P:(c0 + chn) * P])
            wd_t = pools["wd"].tile([P, chn * H], BF, tag="wd")
            rings[(ci + ring_flip) % 2].dma_start(
                wd_t[:].rearrange("p (i h) -> p i h", h=H),
                wd_d.rearrange("(i p) h -> p i h", p=P)[:, c0:c0 + chn, :])

        at_t = at_p.tile([P, CHUNK * n_pairs * S2], BF, tag="aT")
        for il in range(chn):
            ps_g = [psgu.tile([P, S2], F32, tag="ps", name=f"psg{pr}")
                    for pr in range(n_pairs)]
            ps_u = [psgu.tile([P, S2], F32, tag="ps", name=f"psu{pr}")
                    for pr in range(n_pairs)]
            for t in range(HT):
                lhs = wg_t[:, t * wcols + woff + il * P:
                           t * wcols + woff + (il + 1) * P]
                for pr in range(n_pairs):
                    nc.tensor.matmul(ps_g[pr][:], lhs,
                                     xt_t[pr][:, t * S2:(t + 1) * S2],
                                     start=(t == 0), stop=(t == HT - 1))
            for t in range(HT):
                lhs = wu_t[:, t * wcols + woff + il * P:
                           t * wcols + woff + (il + 1) * P]
                for pr in range(n_pairs):
                    nc.tensor.matmul(ps_u[pr][:], lhs,
                                     xt_t[pr][:, t * S2:(t + 1) * S2],
                                     start=(t == 0), stop=(t == HT - 1))
            for pr in range(n_pairs):
                tmp_g = tmp_p.tile([P, S2], F32, tag="tmp")
                nc.scalar.activation(tmp_g[:], ps_g[pr][:], GELU)
                nc.vector.tensor_mul(
                    at_t[:, (il * n_pairs + pr) * S2:
                         (il * n_pairs + pr + 1) * S2],
                    tmp_g[:], ps_u[pr][:])

        # down: weight-stationary, i-outer accumulation in PSUM
        first, last = (ci == 0), (ci == n_chunks - 1)
        wdoff = c0 if res_w is not None else 0
        for htp in range(0, HT, DHT):
            psd = [[psd_p.tile([P, S2], F32, tag="psd", name=f"psd{hl}_{pr}")
                    for pr in range(n_pairs)] for hl in range(DHT)]
            for il in range(chn):
                for hl in range(DHT):
                    ht = htp + hl
                    lhs = wd_t[:, (wdoff + il) * H + ht * P:
                               (wdoff + il) * H + (ht + 1) * P]
                    for pr in range(n_pairs):
                        nc.tensor.matmul(
                            psd[hl][pr][:], lhs,
                            at_t[:, (il * n_pairs + pr) * S2:
                                 (il * n_pairs + pr + 1) * S2],
                            start=(il == 0), stop=(il == chn - 1))
            for hl in range(DHT):
                ht = htp + hl
                for pr in range(n_pairs):
                    col = (ht * n_pairs + pr) * S2
                    if first and last:
                        nc.vector.tensor_copy(stage[:, col:col + S2],
                                              psd[hl][pr][:])
                    elif last:
                        nc.vector.tensor_add(stage[:, col:col + S2],
                                             acc[:, col:col + S2],
                                             psd[hl][pr][:])
                    elif first:
                        nc.vector.tensor_copy(acc[:, col:col + S2],
                                              psd[hl][pr][:])
                    else:
                        nc.vector.tensor_add(acc[:, col:col + S2],
                                             acc[:, col:col + S2],
                                             psd[hl][pr][:])

    sv = stage[:].rearrange("p (t q c) -> p t q c", q=n_pairs, c=S2)
    for pr in range(n_pairs):
        for half in range(2):
            b = 2 * pr + half
            rings[(b + ring_flip) % 2].dma_start(
                out_rows[b].rearrange("(t p) s -> p t s", p=P),
                sv[:, :, pr, half * S:(half + 1) * S])


def _build_kernel(C, nreps=1):
    """C = per-core routed-batch capacity (= 2 * sum of slot pairs)."""
    assert C == 2 * sum(np_ for np_, _ in GROUPS)
    nc = bacc.Bacc("TRN2", target_bir_lowering=False, debug=False,
                   num_devices=NCORES)
    xt_r = nc.dram_tensor("xt_r", [C, H, S], BF, kind="ExternalInput").ap()
    xt_all = nc.dram_tensor("xt_all", [B, H, S], BF, kind="ExternalInput").ap()
    wexp = []
    for gi, (np_, ni_) in enumerate(GROUPS):
        wi = ni_ * P
        wexp.append(tuple(
            nc.dram_tensor(f"w{nm}_{gi}", shp, BF, kind="ExternalInput").ap()
            for nm, shp in (("g", [H, wi]), ("u", [H, wi]), ("d", [wi, H]))))
    wg_s = nc.dram_tensor("wg_s", [H, IS], BF, kind="ExternalInput").ap()
    wu_s = nc.dram_tensor("wu_s", [H, IS], BF, kind="ExternalInput").ap()
    wd_s = nc.dram_tensor("wd_s", [IS, H], BF, kind="ExternalInput").ap()
    out_r = nc.dram_tensor("out_r", [C, H, S], BF, kind="ExternalOutput").ap()
    out_s = nc.dram_tensor("out_s", [B, H, S], BF, kind="ExternalOutput").ap()

    with tile.TileContext(nc) as tc:
        import contextlib
        with contextlib.ExitStack() as ctx:
            pools = {
                "xt": ctx.enter_context(tc.tile_pool(name="xt", bufs=4)),
                "psgu": ctx.enter_context(
                    tc.tile_pool(name="psgu", bufs=4, space="PSUM")),
                "psd": ctx.enter_context(
                    tc.tile_pool(name="psd", bufs=4, space="PSUM")),
                "tmp": ctx.enter_context(tc.tile_pool(name="tmp", bufs=3)),
                "aT": ctx.enter_context(tc.tile_pool(name="aT", bufs=2)),
                "acc": ctx.enter_context(tc.tile_pool(name="acc", bufs=1)),
                "stage": ctx.enter_context(tc.tile_pool(name="stage", bufs=1)),
                "wg": ctx.enter_context(tc.tile_pool(name="wg", bufs=2)),
                "wu": ctx.enter_context(tc.tile_pool(name="wu", bufs=2)),
                "wd": ctx.enter_context(tc.tile_pool(name="wd", bufs=3)),
                "shw": ctx.enter_context(tc.tile_pool(name="shw", bufs=1)),
            }

            nsi = IS // P
            shg = shu = shd = None

            def _load_shared_slabs():
                # shared-expert resident slabs (layout == streamed chunk
                # slabs with chn = IS//P); emitted after the first expert
                # group so its xt/weight DMAs go first at kernel start
                nonlocal shg, shu, shd
                shg = pools["shw"].tile([P, HT * IS], BF, tag="shg")
                shu = pools["shw"].tile([P, HT * IS], BF, tag="shu")
                shd = pools["shw"].tile([P, nsi * H], BF, tag="shd")
                nc.sync.dma_start(
                    shg[:].rearrange("p (t c) -> p t c", c=IS),
                    wg_s.rearrange("(t p) c -> p t c", p=P))
                nc.scalar.dma_start(
                    shu[:].rearrange("p (t c) -> p t c", c=IS),
                    wu_s.rearrange("(t p) c -> p t c", p=P))
                nc.sync.dma_start(
                    shd[:].rearrange("p (i h) -> p i h", h=H),
                    wd_s.rearrange("(i p) h -> p i h", p=P))

            for _rep in range(nreps):
                row = 0
                flip = 0
                for gi, (npair, ni_) in enumerate(GROUPS):
                    rows = list(range(row, row + 2 * npair))
                    _group(nc, pools,
                           [xt_r[r] for r in rows],
                           wexp[gi][0], wexp[gi][1], wexp[gi][2],
                           [out_r[r] for r in rows], npair, ni_,
                           ring_flip=flip)
                    row += 2 * npair
                    flip ^= 1
                    if shg is None:
                        _load_shared_slabs()

                for g in range(B // 4):
                    rows = list(range(4 * g, 4 * g + 4))
                    _group(nc, pools, [xt_all[r] for r in rows],
                           None, None, None,
                           [out_s[r] for r in rows], 2, nsi,
                           res_w=(shg, shu, shd), ring_flip=g % 2)

    nc.compile()
    return nc


_KERNEL_CACHE = {}


def _get_kernel(groups):
    if groups not in _KERNEL_CACHE:
        global GROUPS
        GROUPS = groups
        _KERNEL_CACHE[groups] = _build_kernel(
            2 * sum(np_ for np_, _ in groups))
    return _KERNEL_CACHE[groups]


def build_with_nreps(groups, nreps):
    """Timing helper: same kernel structure, body repeated nreps times."""
    global GROUPS
    GROUPS = groups
    return _build_kernel(2 * sum(np_ for np_, _ in groups), nreps=nreps)


def _routing(router_logits):
    """Replicate reference routing in numpy f32: softmax, top-2, renorm."""
    rl = np.asarray(router_logits, np.float32)
    m = rl.max(axis=-1, keepdims=True)
    ex = np.exp(rl - m, dtype=np.float32)
    rw = ex / ex.sum(axis=-1, keepdims=True)
    sel = np.argsort(-rw, axis=-1, kind="stable")[:, :TOP_K]
    w = np.take_along_axis(rw, sel, axis=-1)
    w = w / w.sum(axis=-1, keepdims=True)
    scale = np.float32(1.0 / NUM_MOE_LAYERS)
    w = scale * w + (np.float32(1.0) - scale) * w
    return sel, w.astype(np.float32)


def kernel(x, router_logits, skill_gate, skill_up, skill_down,
           shared_gate, shared_up, shared_down):
    x = np.asarray(x, np.float32)
    skill_gate = np.asarray(skill_gate, NPBF)
    skill_up = np.asarray(skill_up, NPBF)
    skill_down = np.asarray(skill_down, NPBF)
    shared_gate = np.asarray(shared_gate, NPBF)
    shared_up = np.asarray(shared_up, NPBF)
    shared_down = np.asarray(shared_down, NPBF)

    sel, w = _routing(router_logits)
    lists = [[] for _ in range(E)]
    wmap = np.zeros((B, E), np.float32)
    for b in range(B):
        for k in range(TOP_K):
            e = int(sel[b, k])
            lists[e].append(b)
            wmap[b, e] = w[b, k]

    # decompose each expert's routed batches into weight-stream groups of
    # <=2 pairs; entries are (batch, is_real).  Two-pair groups are assigned
    # to one core each ("own" slots); leftover single pairs become
    # tensor-parallel slots split over I across ALL cores.
    groups2, groups1 = [], []
    for e in range(E):
        ent = [(b, True) for b in lists[e]]
        if len(ent) % 2:
            ent.append((0, False))
        pairs = [ent[i:i + 2] for i in range(0, len(ent), 2)]
        for i in range(0, len(pairs) - 1, 2):
            groups2.append((e, pairs[i] + pairs[i + 1]))
        if len(pairs) % 2:
            groups1.append((e, pairs[-1]))
    n2 = max(1, -(-len(groups2) // NCORES))
    n_tp = len(groups1)
    TPI = I // NCORES  # i-columns per core for a tp slot
    cfg = ((2, I // P),) * n2 + ((1, TPI // P),) * n_tp
    dummy2 = (0, [(0, False)] * 4)
    groups2 += [dummy2] * (n2 * NCORES - len(groups2))

    xt = np.ascontiguousarray(
        x.transpose(0, 2, 1)).astype(NPBF)  # [B, H, S] bf16
    nc = _get_kernel(cfg)

    in_maps = []
    core_slots = []
    for c in range(NCORES):
        own = [groups2[c * n2 + j] for j in range(n2)]
        core_slots.append(own)
        batches = [b for _, ent in own for b, _ in ent]
        batches += [b for _, ent in groups1 for b, _ in ent]
        m = {
            "xt_r": np.ascontiguousarray(xt[batches]),
            "xt_all": xt,
            "wg_s": np.ascontiguousarray(shared_gate[:, c * IS:(c + 1) * IS]),
            "wu_s": np.ascontiguousarray(shared_up[:, c * IS:(c + 1) * IS]),
            "wd_s": np.ascontiguousarray(shared_down[c * IS:(c + 1) * IS, :]),
        }
        for gi, (e, _) in enumerate(own):
            m[f"wg_{gi}"] = skill_gate[e]
            m[f"wu_{gi}"] = skill_up[e]
            m[f"wd_{gi}"] = skill_down[e]
        for tj, (e, _) in enumerate(groups1):
            gi = n2 + tj
            sl = slice(c * TPI, (c + 1) * TPI)
            m[f"wg_{gi}"] = np.ascontiguousarray(skill_gate[e][:, sl])
            m[f"wu_{gi}"] = np.ascontiguousarray(skill_up[e][:, sl])
            m[f"wd_{gi}"] = np.ascontiguousarray(skill_down[e][sl, :])
        in_maps.append(m)

    trace = bool(os.environ.get("TRNK_TRACE"))
    res = run_bass_kernel_spmd(nc, in_maps, core_ids=list(range(NCORES)),
                               trace=trace,
                               trace_cores=list(range(NCORES)) if trace else None)
    kernel.last_exec_time_ns = res.exec_time_ns
    kernel.last_results = res
    kernel.last_nc = nc
    kernel.last_in_maps = in_maps
    kernel.last_cfg = cfg

    out = np.zeros((B, S, H), np.float32)
    n_own_rows = 0
    for c in range(NCORES):
        r = res.results[c]["out_r"]  # [C, H, S] bf16
        row = 0
        for e, ent in core_slots[c]:
            for b, real in ent:
                if real:
                    out[b] += wmap[b, e] * r[row].astype(np.float32).T
                row += 1
        n_own_rows = row
    # tp slots: rows are partial (I-slice) sums — reduce across cores
    for tj, (e, ent) in enumerate(groups1):
        for k, (b, real) in enumerate(ent):
            if real:
                row = n_own_rows + 2 * tj + k
                part = sum(res.results[c]["out_r"][row].astype(np.float32)
                           for c in range(NCORES))
                out[b] += wmap[b, e] * part.T
    for c in range(NCORES):
        out += res.results[c]["out_s"].astype(np.float32).transpose(0, 2, 1)
    return out
